# revision 3
# baseline (speedup 1.0000x reference)
"""Winograd F(2)-phase Trainium2 kernel for strided Conv2d + stride-perm + bias.

Stride-2 3x3 conv decomposes into 4 stride-1 phase convs on parity planes:
  ee: 1x1, eo: 1x2, oe: 2x1, oo: 2x2.
Winograd F(2,2) applied per dim maps ALL phases into one shared 3x3 grid of
PSUM product planes q[A,T] (per half: 9 x [8,16] f32), where the output
combine is y(dr,dc) = q[0,0] + q[0,1+dc] + q[1+dr,0] + q[1+dr,1+dc] + bias.

Input transforms (B^T x B combos) and weight transforms (G w G^T) are done
on the HOST; the device sees 25 pre-transformed [16,16] planes per
(img, ci-tile) and 16 transformed weight matrices per (cit, cot). Every
matmul is a contiguous [8,16]=128-col block accumulating into one q region.
PE columns drop 30.6% vs direct (51200 vs 73728 per rep per core).

Eviction per half: 2 DVE tensor_adds (PSUM->SBUF f16 C[A,dc] = q[A,0] +
q[A,1+dc]) + 2 DVE scalar_tensor_tensor ops writing the stride-permuted
output layout [si,sj,i,j] directly (bias folded in), so the store DMA
stays fully contiguous.

Data-parallel over batch: 2 images/core on 8 cores.
"""

import os
import time

import numpy as np

_B, _C = 16, 256
_NCORES = 8
_IMGS = _B // _NCORES

# plane index layout per (img, cit): 0..8 oo d[A*3+T], 9..12 ee e[dr*2+dc],
# 13..18 eo f[t*2+dr], 19..24 oe g[t*2+dc]
# weight index layout per (cit, cot): 0..8 oo, 9 ee, 10..12 eo, 13..15 oe


def _mm_plan():
    # widx -> list of (plane, region A*3+T); ordered so each PSUM bank's
    # first matmul (start=True, clears the bank's has_written bits) comes
    # before any other matmul touching that bank.
    plan = {}
    for A in range(3):
        for T in range(3):
            plan[A * 3 + T] = [(A * 3 + T, A * 3 + T)]
    plan[9] = [
        (9 + dr * 2 + dc, (1 + dr) * 3 + (1 + dc))
        for dr in range(2)
        for dc in range(2)
    ]
    for t in range(3):
        plan[10 + t] = [(13 + t * 2 + dr, (1 + dr) * 3 + t) for dr in range(2)]
    for t in range(3):
        plan[13 + t] = [(19 + t * 2 + dc, t * 3 + (1 + dc)) for dc in range(2)]
    order = [0, 4, 8, 1, 2, 3, 5, 6, 7, 9, 10, 11, 12, 13, 14, 15]
    return [(w, plan[w]) for w in order]


_PLAN = _mm_plan()

_PROG_CACHE = {}


def _build_program(reps: int):
    import concourse.tile as tile
    from concourse import bacc, mybir

    f32 = mybir.dt.float32
    f16 = mybir.dt.float16
    ADD = mybir.AluOpType.add

    nc = bacc.Bacc("TRN2", target_bir_lowering=False, debug=False)

    xph = nc.dram_tensor(
        "xph", [_IMGS, 2, 128, 25, 16, 16], f16, kind="ExternalInput"
    ).ap()
    wt = nc.dram_tensor("wt", [128, 8192], f16, kind="ExternalInput").ap()
    bs = nc.dram_tensor("bs", [128, 2], f32, kind="ExternalInput").ap()
    out = nc.dram_tensor("out", [_IMGS, 2, 128, 1024], f32, kind="ExternalOutput").ap()

    with tile.TileContext(nc) as tc:
        with (
            tc.tile_pool(name="const", bufs=1) as constp,
            tc.tile_pool(name="xbuf", bufs=1) as xp,
            tc.tile_pool(name="cbuf", bufs=2) as cp,
            tc.tile_pool(name="obuf", bufs=2) as obp,
            tc.tile_pool(name="psum", bufs=2, space="PSUM") as psp,
        ):
            wtile = constp.tile([128, 8192], f16)
            btile = constp.tile([128, 2], f32)
            xt = {}
            for img in range(_IMGS):
                for cit in range(2):
                    xt[(img, cit)] = xp.tile(
                        [128, 25, 16, 16],
                        f16,
                        tag=f"x_{img}_{cit}",
                        name=f"x_{img}_{cit}",
                    )

            _eng = [nc.sync, nc.scalar]
            _ei = [0]

            def _dma(dst, src):
                _eng[_ei[0] & 1].dma_start(dst, src)
                _ei[0] += 1

            # one-time loads: weights per (cit,cot) chunk; x per (img,cit)
            _dma(wtile[:, 0:2048], wt[:, 0:2048])
            _dma(xt[(0, 0)][:], xph[0, 0])
            _dma(wtile[:, 4096:6144], wt[:, 4096:6144])
            _dma(xt[(0, 1)][:], xph[0, 1])
            _dma(btile[:], bs[:])
            _dma(wtile[:, 2048:4096], wt[:, 2048:4096])
            _dma(wtile[:, 6144:8192], wt[:, 6144:8192])
            _dma(xt[(1, 0)][:], xph[1, 0])
            _dma(xt[(1, 1)][:], xph[1, 1])

            for _rep in range(reps):
                for img in range(_IMGS):
                    for cot in range(2):
                        ob = obp.tile([128, 2, 2, 16, 16], f32, tag="ob", name="ob")
                        for half in range(2):
                            qt = psp.tile([128, 12, 8, 16], f32, tag="q", name="q")
                            mms = [
                                (cit, widx, p, reg)
                                for cit in range(2)
                                for widx, lst in _PLAN
                                for p, reg in lst
                            ]
                            started = set()
                            for i, (cit, widx, p, reg) in enumerate(mms):
                                bank = reg // 4
                                st = bank not in started
                                started.add(bank)
                                rhs = xt[(img, cit)][
                                    :, p, half * 8 : half * 8 + 8, :
                                ]
                                s = ((cit * 2 + cot) * 16 + widx) * 128
                                nc.tensor.matmul(
                                    qt[:, reg],
                                    wtile[:, s : s + 128],
                                    rhs,
                                    start=st,
                                    stop=(i == len(mms) - 1),
                                    skip_group_check=True,
                                )
                            # combine stage 1: C[A,dc] = q[A,0] + q[A,1+dc].
                            # DVE has one PSUM read port, so q[A,0] goes
                            # through SBUF via ScalarE first.
                            st = cp.tile([128, 3, 8, 16], f32, tag="S", name="S")
                            nc.scalar.copy(st[:], qt[:, 0:9:3])
                            ct = cp.tile([128, 3, 2, 8, 16], f16, tag="C", name="C")
                            qv = qt.rearrange("p (A T) v u -> p A T v u", A=4)
                            nc.vector.tensor_add(
                                ct[:],
                                st[:, :, None].broadcast_to([128, 3, 2, 8, 16]),
                                qv[:, 0:3, 1:3],
                            )
                            # combine stage 2 (+bias), direct into permuted
                            # output quadrants: y = (C[0] + bias) + C[1+dr]
                            for dr in range(2):
                                nc.vector.scalar_tensor_tensor(
                                    ob[:, dr, :, half * 8 : half * 8 + 8, :],
                                    ct[:, 0],
                                    btile[:, cot : cot + 1],
                                    ct[:, 1 + dr],
                                    op0=ADD,
                                    op1=ADD,
                                )
                        nc.sync.dma_start(out[img, cot], ob[:])

    nc.compile()
    return nc


def _get_program(reps: int):
    if reps not in _PROG_CACHE:
        _PROG_CACHE[reps] = _build_program(reps)
    return _PROG_CACHE[reps]


def _prep_inputs(x, weight, bias):
    x = np.ascontiguousarray(np.asarray(x, dtype=np.float32))
    weight = np.ascontiguousarray(np.asarray(weight, dtype=np.float32))
    bias = np.ascontiguousarray(np.asarray(bias, dtype=np.float32))

    Xp = np.pad(x, ((0, 0), (0, 0), (1, 0), (1, 0)))  # [B,C,65,65]
    Pee = Xp[:, :, 1:64:2, 1:64:2]  # [32,32]
    Peo = Xp[:, :, 1:64:2, 0:65:2]  # [32,33]
    Poe = Xp[:, :, 0:65:2, 1:64:2]  # [33,32]
    Poo = Xp[:, :, 0:65:2, 0:65:2]  # [33,33]

    planes = np.empty((_B, _C, 25, 16, 16), np.float32)
    # oo: col transform then row transform
    C0 = Poo[:, :, :, 1:32:2]
    C1 = Poo[:, :, :, 0:31:2] - C0
    C2 = Poo[:, :, :, 2:33:2] - C0
    for T, Ct in enumerate((C0, C1, C2)):
        d0 = Ct[:, :, 1:32:2]
        planes[:, :, 0 * 3 + T] = d0
        planes[:, :, 1 * 3 + T] = Ct[:, :, 0:31:2] - d0
        planes[:, :, 2 * 3 + T] = Ct[:, :, 2:33:2] - d0
    for dr in range(2):
        for dc in range(2):
            planes[:, :, 9 + dr * 2 + dc] = Pee[:, :, dr::2, dc::2]
    T0 = Peo[:, :, :, 1:32:2]
    T1 = Peo[:, :, :, 0:31:2] - T0
    T2 = Peo[:, :, :, 2:33:2] - T0
    for t, Tt in enumerate((T0, T1, T2)):
        for dr in range(2):
            planes[:, :, 13 + t * 2 + dr] = Tt[:, :, dr::2, :]
    N0 = Poe[:, :, 1:32:2, :]
    N1 = Poe[:, :, 0:31:2, :] - N0
    N2 = Poe[:, :, 2:33:2, :] - N0
    for t, Nt in enumerate((N0, N1, N2)):
        for dc in range(2):
            planes[:, :, 19 + t * 2 + dc] = Nt[:, :, :, dc::2]

    xph_all = planes.astype(np.float16).reshape(_B, 2, 128, 25, 16, 16)

    # weight transforms -> [co, ci, 16]
    wtf = np.empty((256, 256, 16), np.float32)
    w00, w02 = weight[:, :, 0, 0], weight[:, :, 0, 2]
    w20, w22 = weight[:, :, 2, 0], weight[:, :, 2, 2]
    R = [(w00 + w20, w02 + w22), (w00, w02), (w20, w22)]
    for A in range(3):
        r0, r1 = R[A]
        wtf[:, :, A * 3 + 0] = r0 + r1
        wtf[:, :, A * 3 + 1] = r0
        wtf[:, :, A * 3 + 2] = r1
    wtf[:, :, 9] = weight[:, :, 1, 1]
    w10, w12 = weight[:, :, 1, 0], weight[:, :, 1, 2]
    wtf[:, :, 10], wtf[:, :, 11], wtf[:, :, 12] = w10 + w12, w10, w12
    w01, w21 = weight[:, :, 0, 1], weight[:, :, 2, 1]
    wtf[:, :, 13], wtf[:, :, 14], wtf[:, :, 15] = w01 + w21, w01, w21

    # [co,ci,widx] -> [cip][cit, cot, widx, cop]
    w5 = wtf.reshape(2, 128, 2, 128, 16)  # [cot, cop, cit, cip, widx]
    wt = np.ascontiguousarray(
        w5.transpose(3, 2, 0, 4, 1).reshape(128, 8192).astype(np.float16)
    )

    bs = np.ascontiguousarray(bias.reshape(2, 128).T)  # [cop, cot]

    in_maps = []
    for c in range(_NCORES):
        in_maps.append(
            {
                "xph": np.ascontiguousarray(xph_all[c * _IMGS : (c + 1) * _IMGS]),
                "wt": wt,
                "bs": bs,
            }
        )
    return in_maps


class _Runner:
    """Persistent jitted SPMD executor for one built program."""

    def __init__(self, nc):
        import jax
        import numpy as _np
        from jax.sharding import Mesh, NamedSharding, PartitionSpec
        from jax.experimental.shard_map import shard_map
        import concourse.mybir as mybir
        from concourse import bass2jax

        bass2jax.install_neuronx_cc_hook()
        self.jax = jax
        self.nc = nc

        partition_name = (
            nc.partition_id_tensor.name if nc.partition_id_tensor else None
        )
        in_names, out_names, out_avals, zero_outs = [], [], [], []
        for alloc in nc.m.functions[0].allocations:
            if not isinstance(alloc, mybir.MemoryLocationSet):
                continue
            name = alloc.memorylocations[0].name
            if alloc.kind == "ExternalInput":
                if name != partition_name:
                    in_names.append(name)
            elif alloc.kind == "ExternalOutput":
                shape = tuple(alloc.tensor_shape)
                dtype = mybir.dt.np(alloc.dtype)
                out_names.append(name)
                out_avals.append(jax.core.ShapedArray(shape, dtype))
                zero_outs.append(_np.zeros(shape, dtype))
        self.in_names = in_names
        self.out_names = out_names
        self.out_avals = out_avals
        self.zero_outs = zero_outs
        n_params = len(in_names)

        def _body(*args):
            operands = list(args)
            if partition_name is not None:
                operands.append(bass2jax.partition_id_tensor())
            outs = bass2jax._bass_exec_p.bind(
                *operands,
                out_avals=tuple(out_avals),
                in_names=tuple(in_names + out_names + ([partition_name] if partition_name else [])),
                out_names=tuple(out_names),
                lowering_input_output_aliases=(),
                sim_require_finite=True,
                sim_require_nnan=True,
                nc=nc,
            )
            return tuple(outs)

        devices = jax.devices()[:_NCORES]
        self.mesh = Mesh(np.asarray(devices), ("core",))
        self.spec = NamedSharding(self.mesh, PartitionSpec("core"))
        n_outs = len(out_names)
        in_specs = (PartitionSpec("core"),) * (n_params + n_outs)
        out_specs = (PartitionSpec("core"),) * n_outs
        self.fn = jax.jit(
            shard_map(
                _body,
                mesh=self.mesh,
                in_specs=in_specs,
                out_specs=out_specs,
                check_rep=False,
            ),
            keep_unused=True,
        )

    def place_inputs(self, in_maps):
        concat = [
            np.concatenate([np.asarray(m[name]) for m in in_maps], axis=0)
            for name in self.in_names
        ]
        return [self.jax.device_put(a, self.spec) for a in concat]

    def place_zeros(self):
        return [
            self.jax.device_put(
                np.zeros((_NCORES * z.shape[0], *z.shape[1:]), z.dtype), self.spec
            )
            for z in self.zero_outs
        ]

    def __call__(self, dev_inputs, dev_zeros):
        outs = self.fn(*dev_inputs, *dev_zeros)
        self.jax.block_until_ready(outs)
        return outs


_RUNNER_CACHE = {}


def _get_runner(reps: int) -> "_Runner":
    if reps not in _RUNNER_CACHE:
        _RUNNER_CACHE[reps] = _Runner(_get_program(reps))
    return _RUNNER_CACHE[reps]


def _run(in_maps, reps: int):
    r = _get_runner(reps)
    dev_in = r.place_inputs(in_maps)
    dev_z = r.place_zeros()
    t0 = time.perf_counter()
    outs = r(dev_in, dev_z)
    dt = time.perf_counter() - t0
    full = np.asarray(outs[0]).reshape(_NCORES, _IMGS, 2, 128, 1024)
    return full.reshape(_B, _C, 1024), dt


def kernel(x, weight, bias):
    in_maps = _prep_inputs(x, weight, bias)
    reps = int(os.environ.get("BASS_CONV_REPS", "1"))
    out, _ = _run(in_maps, reps)
    return out
